# revision 33
# baseline (speedup 1.0000x reference)
"""Multi-head attention with KV cache, sharded over 8 NeuronCores by head.

Problem (hardcoded shapes):
  x       [4, 512, 1024]      hidden states (B, T, D)
  k_prev  [4, 16, 3584, 64]   KV cache (B, H, PAST, HD)
  v_prev  [4, 16, 3584, 64]
  Wq/Wk/Wv/Wo [1024, 1024]    projection weights (torch Linear: y = x @ W.T)

Sharding: 16 heads / 8 cores = 2 heads per core (data stays full along batch).
Each core computes q/k/v projections for its 2 heads (column-parallel),
full attention for its heads, and a column-parallel o_proj partial
[2048, 1024]; the host sums the 8 partials (the o_proj all-reduce).

Device algorithm per core:
  ScalarE (exp over every live [key, query] score: ~16M elements/core at
  128 lanes x 1.2 GHz ~= 122 us) is the bottleneck engine, so the whole
  batch loop is one software pipeline that keeps it streaming:
  - work item = (head, query chunk of 128, balanced group of ~10 key
    chunks).  Per item: bf16 scores k @ q^T into a 3-bank PSUM tile (only
    causally-live chunks; the diagonal chunk gets a shared [128,128]
    lower-triangle mask added via identity @ mask), one exp -> bf16 p^T,
    then P @ V accumulation matmuls.
  - P @ V runs transposed: acc[q(128 part), hd+1] accumulates over key
    chunks (65-col matmuls, half the PE column cost of the [hd, q]
    orientation); rhs = v chunk | 1, so the softmax denominator lands in
    col 64 and the divide is a per-partition reciprocal + tensor_scalar.
    The 4 query chunks time-share one PSUM bank as *sequential*
    accumulation groups (the backend allows only one open accumulation
    group per bank - interleaved groups corrupt each other).
  - pipeline beat: exp(i) | scores(i+2) | AV(i-4).  Scores stay 2 tiles
    ahead of ScalarE (scp bufs=2), AV lags 4 items (pT bufs=8) so thunk
    hiccups never stall the exp stream; each batch's trailing AVs are
    carried past the next batch's first two score items.
  - projection (bf16) / KV-append / v-transpose / o_proj work is chopped
    into small thunks drained 2-3 per beat through the PE slack, with
    emission-order barriers before their consumers' batches (thunks must
    be *emitted* before any consumer instruction - program order defines
    dependencies).
  - cold start: PE warmup matmuls ramp the p-state while DMAs land in
    q-path-first order; batch 0 emits q-proj inline, k-proj + cache-append
    at beat 0 between the first two score items, v-proj via beats.
  - tail: last batch's o_proj rotates PSUM through flex + both score
    buffers with staging copies split DVE/ScalarE; out is bf16 (host
    upconverts while summing the 8 o_proj partials in float64).
"""

import numpy as np
import ml_dtypes

import concourse.bass as bass
import concourse.mybir as mybir
import concourse.tile as tile
from concourse import bacc
from concourse.bass_utils import run_bass_kernel_spmd
from concourse.masks import make_identity

B, T, D = 4, 512, 1024
H, HD = 16, 64
PAST = 3584
L = PAST + T            # 4096 == MAX_CACHE, nothing is trimmed
SCALE = 1.0 / np.sqrt(HD).astype(np.float32)
NCORES = 8
HPC = H // NCORES       # heads per core = 2
TOK = B * T             # 2048
NCH = L // 128          # 32 key chunks per (b, h)
PCH = PAST // 128       # 28 chunks from the cache
QCH = T // 128          # 4 query chunks
GCH = 12                # key chunks per PSUM scores group (3 banks)
FP32 = mybir.dt.float32
FP32R = mybir.dt.float32r
BF16 = mybir.dt.bfloat16
NEG = -1.0e30

_cache = {}


def _build():
    nc = bacc.Bacc(None, target_bir_lowering=False)

    xT = nc.dram_tensor("xT", [D, TOK], BF16, kind="ExternalInput")
    wqT = nc.dram_tensor("wqT", [128, D // 128, 128], BF16,
                         kind="ExternalInput")
    wkT = nc.dram_tensor("wkT", [128, D // 128, 128], BF16,
                         kind="ExternalInput")
    wvT = nc.dram_tensor("wvT", [128, D // 128, 128], BF16,
                         kind="ExternalInput")
    woT = nc.dram_tensor("woT", [128, D], FP32R, kind="ExternalInput")
    kTp = nc.dram_tensor("kTp", [B, 128, PAST], BF16, kind="ExternalInput")
    vp = nc.dram_tensor("vp", [B, 128, HPC, PCH, HD + 1], BF16, kind="ExternalInput")
    out = nc.dram_tensor("out", [TOK, D], BF16, kind="ExternalOutput")

    Exp = mybir.ActivationFunctionType.Exp
    mult = mybir.AluOpType.mult

    with tile.TileContext(nc) as tc:
        with (
            tc.tile_pool(name="const", bufs=1) as const,
            tc.tile_pool(name="persist", bufs=1) as persist,
            tc.tile_pool(name="kv", bufs=2) as kv,
            tc.tile_pool(name="pt", bufs=8) as ptp,
            tc.tile_pool(name="div", bufs=2) as divp,
            tc.tile_pool(name="osb", bufs=2) as osbp,
            tc.tile_pool(name="stage", bufs=2) as stage,
            tc.tile_pool(name="acc_ps", bufs=1, space="PSUM") as accp,
            tc.tile_pool(name="flex_ps", bufs=1, space="PSUM") as flexp,
        ):
            # ---- constants ----
            identity = const.tile([128, 128], FP32)
            make_identity(nc, identity)
            identity_b = const.tile([128, 128], BF16)
            nc.vector.tensor_copy(identity_b, identity)
            # shared lower-triangle mask for the diagonal key chunks:
            # tri[key j, query i] = 0 where i >= j else NEG
            tri_f = const.tile([128, 128], FP32)
            nc.gpsimd.memset(tri_f, 0.0)
            nc.gpsimd.affine_select(
                out=tri_f, in_=tri_f, compare_op=mybir.AluOpType.is_ge,
                fill=NEG, base=0, channel_multiplier=-1,
                pattern=[[1, 128]],
            )
            tri_b = const.tile([128, 128], BF16)
            nc.vector.tensor_copy(tri_b, tri_f)

            ones_c = const.tile([128, 1], FP32)
            nc.gpsimd.memset(ones_c, 1.0)
            warm = const.tile([1, 1], FP32)
            nc.scalar.activation(warm, ones_c[:1, :], Exp)

            wrhs = const.tile([128, 512], BF16)
            nc.gpsimd.memset(wrhs, 0.0)

            def pe_warmup(n):
                # dummy back-to-back matmuls on resident constants: ramps
                # the tensor engine's p-state before real work arrives
                wtile = flexp.tile([128, 512], FP32, tag="flex",
                                   name="warmps")
                for i in range(n):
                    nc.tensor.matmul(
                        wtile, lhsT=identity_b, rhs=wrhs,
                        start=True, stop=True,
                        skip_group_check=True,
                    )

            # ---- persistent SBUF ----
            woT_s = persist.tile([128, D], FP32R)
            qT = persist.tile([128, TOK], BF16, tag="qT")
            kTn = persist.tile([128, TOK], BF16, tag="kTn")
            vTn = persist.tile([128, TOK], FP32, tag="vTn")
            oT = persist.tile([128, TOK], FP32R, tag="oT")

            with (
                tc.tile_pool(name="xw", bufs=1) as xw,
                tc.tile_pool(name="xs", bufs=2) as xs,
                tc.tile_pool(name="sc_ps", bufs=2, space="PSUM") as scp,
            ):
                xT_r = xT.rearrange("(ko p) t -> p ko t", p=128)

                # ---- background thunk machinery ----
                # bg_pre: proj/KV-setup thunks; MUST all be emitted before
                # the attention batch that reads their outputs (barrier at
                # attn_batch start).  bg_post: o_proj thunks, no such
                # hazard -- drained through leftover beat slack.
                bg_pre = []      # entries: (due_batch, thunk)
                bg_post = []

                def drain_bg(n=1):
                    for _ in range(n):
                        if bg_pre:
                            bg_pre.pop(0)[1]()
                        elif bg_post:
                            bg_post.pop(0)()

                def flush_pre(due):
                    rest = []
                    for d, f in bg_pre:
                        if d <= due:
                            f()
                        else:
                            rest.append((d, f))
                    bg_pre[:] = rest

                def enq_proj(tcn, xT_s=None):
                    """DMA x chunk now; enqueue 3 proj psum groups as
                    thunks (4x 2-matmul + 1 copy each)."""
                    if xT_s is None:
                        xT_s = xs.tile(
                            [128, D // 128, 512], BF16, tag="xT",
                            name=f"xT_s{tcn}"
                        )
                        half = D // 256
                        nc.sync.dma_start(
                            xT_s[:, :half, :], xT_r[:, :half, bass.ts(tcn, 512)]
                        )
                        nc.sync.dma_start(
                            xT_s[:, half:, :], xT_r[:, half:, bass.ts(tcn, 512)]
                        )
                    for name, dst in (("q", qT), ("k", kTn), ("v", vTn)):
                        box = {}

                        def mk_mm(name, ko, box):
                            def f():
                                if "ps" not in box:
                                    box["ps"] = flexp.tile(
                                        [128, 512], FP32, tag="flex",
                                        name="flexp"
                                    )
                                nc.tensor.matmul(
                                    box["ps"],
                                    lhsT=w_s[name][:, ko, :],
                                    rhs=xT_s[:, ko, :],
                                    start=(ko == 0),
                                    stop=(ko == D // 128 - 1),
                                )
                            return f

                        def mk_cp(dst, box, lo, hi):
                            def f():
                                nc.vector.tensor_copy(
                                    dst[:, tcn * 512 + lo : tcn * 512 + hi],
                                    box["ps"][:, lo:hi]
                                )
                            return f

                        for ko in range(D // 128):
                            bg_pre.append((tcn, mk_mm(name, ko, box)))
                        if tcn == 0 and name in ("q", "k"):
                            # first query chunk lands first: unblocks the
                            # cold-start scores a copy earlier
                            bg_pre.append((tcn, mk_cp(dst, box, 0, 128)))
                            bg_pre.append((tcn, mk_cp(dst, box, 128, 512)))
                        else:
                            bg_pre.append((tcn, mk_cp(dst, box, 0, 512)))

                def enq_setup(b, kT=None, vac=None, inline_kcopy=False):
                    """DMA next batch's KV now; enqueue the cache-append
                    copy + v transposes as thunks."""
                    if kT is None:
                        kT = kv.tile([128, L], BF16, tag="kT", name=f"kT{b}")
                        nc.sync.dma_start(kT[:, :PAST], kTp[b, :, :])
                    if vac is None:
                        vac = kv.tile(
                            [128, HPC, PCH, HD + 1], BF16, tag="vac",
                            name=f"vac{b}"
                        )
                        nc.sync.dma_start(vac, vp[b, :, :, :, :])
                    van = kv.tile([128, HPC, QCH, HD + 1], BF16, tag="van",
                                  name=f"van{b}")
                    def kcopy_a():
                        nc.vector.tensor_copy(
                            kT[:, PAST : PAST + 128],
                            kTn[:, b * T : b * T + 128])

                    def kcopy_b():
                        nc.vector.tensor_copy(
                            kT[:, PAST + 128 :],
                            kTn[:, b * T + 128 : (b + 1) * T])
                    ret_kcopy = None
                    if inline_kcopy == "defer":
                        ret_kcopy = (kcopy_a, kcopy_b)
                    elif inline_kcopy:
                        kcopy_a(); kcopy_b()
                    else:
                        bg_pre.append((b, kcopy_a))
                        bg_pre.append((b, kcopy_b))
                    bg_pre.append((b, lambda: nc.gpsimd.memset(
                        van[:, :, :, HD], 1.0)))
                    for h in range(HPC):
                        hsl = slice(h * HD, (h + 1) * HD)
                        for tt in range(QCH):
                            def mk_tr(hsl, h, tt):
                                def f():
                                    tp = flexp.tile([128, 512], FP32,
                                                    tag="flex", name="flexv")
                                    nc.tensor.transpose(
                                        tp[:, :HD],
                                        vTn[hsl, b * T + tt * 128
                                            : b * T + (tt + 1) * 128],
                                        identity[hsl, hsl],
                                    )
                                    nc.vector.tensor_copy(
                                        van[:, h, tt, :HD], tp[:, :HD]
                                    )
                                return f
                            bg_pre.append((b, mk_tr(hsl, h, tt)))
                    return kT, (vac, van), ret_kcopy

                def enq_oproj(b, o_sb):
                    """o^T transposes + o_proj partial for batch b (drained
                    during batch b+1's pipeline)."""
                    for qq in range(QCH):
                        def mk_tr(qq):
                            def f():
                                tp = flexp.tile([128, 512], FP32, tag="flex",
                                                name="flexo")
                                nc.tensor.transpose(
                                    tp[:, :128], o_sb[:, qq, :], identity
                                )
                                nc.vector.tensor_copy(
                                    oT[:, b * T + qq * 128
                                       : b * T + (qq + 1) * 128],
                                    tp[:, :128],
                                )
                            return f
                        bg_post.append(mk_tr(qq))
                    out_r = out[bass.ts(b, T), :].rearrange(
                        "(tt p) d -> p tt d", p=128)
                    ost = stage.tile([128, T // 128, D], BF16, tag="ost",
                                     name=f"ost{b}")
                    for tt in range(T // 128):
                        tsl = slice(b * T + tt * 128, b * T + (tt + 1) * 128)
                        box = {}

                        def mk_mm(tt, tsl, nh, box):
                            def f():
                                ps = flexp.tile([128, 512], FP32, tag="flex",
                                                name="flexm")
                                nc.tensor.matmul(
                                    ps,
                                    lhsT=oT[:, tsl],
                                    rhs=woT_s[:, bass.ts(nh, 512)],
                                    start=True,
                                    stop=True,
                                )
                                box[nh] = ps
                            return f

                        def mk_cp(tt, nh, box):
                            def f():
                                nc.vector.tensor_copy(
                                    ost[:, tt, bass.ts(nh, 512)], box[nh]
                                )
                                if nh == 1:
                                    nc.sync.dma_start(
                                        out_r[:, tt, :], ost[:, tt, :]
                                    )
                            return f

                        for nh in range(2):
                            bg_post.append(mk_mm(tt, tsl, nh, box))
                            bg_post.append(mk_cp(tt, nh, box))

                # ---- weight / first-batch loads ----
                # DMA order drives the cold start: q path (wq, x chunk 0)
                # first, then k cache + wk so the first exps stream while
                # v / o-weights land
                pe_warmup(8)
                w_s = {}
                for name, w in (("q", wqT), ("k", wkT), ("v", wvT)):
                    w_s[name] = xw.tile(
                        [128, D // 128, 128], BF16, tag=f"w{name}",
                        name=f"w{name}"
                    )
                nc.sync.dma_start(w_s["q"], wqT[:, :, :])
                xT_s0 = xs.tile([128, D // 128, 512], BF16, tag="xT",
                                name="xT_s0")
                half = D // 256
                nc.sync.dma_start(xT_s0[:, :half, :], xT_r[:, :half, :512])
                nc.sync.dma_start(xT_s0[:, half:, :], xT_r[:, half:, :512])
                kT0 = kv.tile([128, L], BF16, tag="kT", name="kT0")
                nc.sync.dma_start(kT0[:, : 12 * 128], kTp[0, :, : 12 * 128])
                nc.sync.dma_start(kT0[:, 12 * 128 : PAST], kTp[0, :, 12 * 128 :])
                nc.sync.dma_start(w_s["k"][:, :4, :], wkT[:, :4, :])
                nc.sync.dma_start(w_s["k"][:, 4:, :], wkT[:, 4:, :])
                nc.sync.dma_start(w_s["v"], wvT[:, :, :])
                vac0 = kv.tile(
                    [128, HPC, PCH, HD + 1], BF16, tag="vac", name="vac0"
                )
                nc.sync.dma_start(vac0[:, 0, :, :], vp[0, :, 0, :, :])
                nc.sync.dma_start(vac0[:, 1, :, :], vp[0, :, 1, :, :])
                nc.sync.dma_start(woT_s, woT[:, :])

                # prologue: q and k projections + cache-append inline (the
                # first scores read them); v projection and the v-transposes
                # drain through batch 0's beats
                enq_proj(0, xT_s=xT_s0)
                # emit only the q projection inline; the k projection +
                # cache-append run at beat 0 of batch 0, AFTER the first two
                # score items, so the first exps start as early as possible
                for _ in range(D // 128 + 2):
                    bg_pre.pop(0)[1]()
                k_thunks = [bg_pre.pop(0)[1] for _ in range(D // 128 + 2)]
                kT0_, kv0, kcopy0 = enq_setup(0, kT=kT0, vac=vac0,
                                              inline_kcopy="defer")
                nxt = (kT0_, kv0)
                # [mms, cp(0:128), kcopy(chunk 28), cp(128:512), kcopy rest]
                k_thunks = (k_thunks[:-1] + [kcopy0[0]]
                            + k_thunks[-1:] + [kcopy0[1]])
                enq_proj(1)

                # ---- the attention pipeline ----
                def attn_batch(b, kT, vac, van, o_sb, hook0=None,
                               carry=None):
                    items = []
                    for h in range(HPC):
                        for qq in range(QCH):
                            nch_q = PCH + qq + 1
                            ngr = (nch_q + GCH - 1) // GCH
                            szs = [nch_q // ngr + (1 if i < nch_q % ngr
                                                   else 0)
                                   for i in range(ngr)]
                            gs, s0 = [], 0
                            for sz in szs:
                                gs.append(list(range(s0, s0 + sz)))
                                s0 += sz
                            for gi, g in enumerate(gs):
                                items.append(
                                    (h, qq, g, gi == len(gs) - 1))
                    ps_t = [None] * len(items)
                    pT_t = [None] * len(items)
                    accs = {}

                    def emit_scores(i):
                        h, qq, g, _ = items[i]
                        hsl = slice(h * HD, (h + 1) * HD)
                        qTh = qT[hsl, b * T + qq * 128
                                 : b * T + (qq + 1) * 128]
                        ps = scp.tile([128, GCH * 128], FP32, tag="sc",
                                      name="scps")
                        ps_t[i] = ps
                        for j, cc in enumerate(g):
                            diag = cc == PCH + qq
                            nc.tensor.matmul(
                                ps[:, bass.ts(j, 128)],
                                lhsT=kT[hsl, bass.ts(cc, 128)],
                                rhs=qTh,
                                start=True,
                                stop=not diag,
                            )
                            if diag:
                                nc.tensor.matmul(
                                    ps[:, bass.ts(j, 128)],
                                    lhsT=identity_b,
                                    rhs=tri_b,
                                    start=False,
                                    stop=True,
                                    skip_group_check=True,
                                )

                    def emit_exp(i):
                        h, qq, g, _ = items[i]
                        ng = len(g)
                        pT = ptp.tile([128, GCH * 128], BF16, tag="pT",
                                      name="pTt")
                        pT_t[i] = pT
                        nc.scalar.activation(
                            pT[:, : ng * 128], ps_t[i][:, : ng * 128], Exp
                        )

                    def emit_av(i):
                        h, qq, g, last = items[i]
                        hosl = slice(h * HD, (h + 1) * HD)
                        if h not in accs:
                            accs[h] = accp.tile([128, QCH, 128], FP32,
                                                tag="acc", name="acct")
                        acc = accs[h]
                        pT = pT_t[i]
                        for j, cc in enumerate(g):
                            vrhs = (vac[:, h, cc, :] if cc < PCH
                                    else van[:, h, cc - PCH, :])
                            nc.tensor.matmul(
                                acc[:, qq, : HD + 1],
                                lhsT=pT[:, bass.ts(j, 128)],
                                rhs=vrhs,
                                start=(cc == 0),
                                stop=(cc == PCH + qq),
                                skip_group_check=True,
                            )
                        if last:
                            r = divp.tile([128, 1], FP32, tag="r", name="rt")
                            nc.vector.reciprocal(
                                r, acc[:, qq, HD : HD + 1])
                            nc.vector.tensor_scalar(
                                out=o_sb[:, qq, hosl],
                                in0=acc[:, qq, :HD],
                                scalar1=r,
                                scalar2=None,
                                op0=mult,
                            )
                            if b == B - 1 and h == HPC - 1:
                                # drain o^T for the final o_proj early
                                tp = flexp.tile([128, 512], FP32,
                                                tag="flex", name="flexq")
                                nc.tensor.transpose(
                                    tp[:, :128], o_sb[:, qq, :], identity)
                                nc.vector.tensor_copy(
                                    oT[:, b * T + qq * 128
                                       : b * T + (qq + 1) * 128],
                                    tp[:, :128])

                    LAG = 3
                    emit_scores(0)
                    if len(items) > 1:
                        emit_scores(1)
                    if carry:
                        for f in carry:
                            f()
                    for i in range(len(items)):
                        emit_exp(i)
                        if i == 0 and hook0:
                            for f in hook0:
                                f()
                        if i + 2 < len(items):
                            emit_scores(i + 2)
                        trail = LAG if b == B - 1 else LAG + 2
                        if i >= LAG and i - LAG < len(items) - trail:
                            j = i - LAG
                            if items[j][2][-1] >= PCH:
                                flush_pre(b)  # AV reads van/kT-append
                            emit_av(j)
                        if i < len(items) - 2:
                            if b == 0:
                                if i >= 1:
                                    drain_bg(3 if len(bg_pre) > 25 else 2)
                            else:
                                drain_bg(2)
                    deferred = []
                    for j in range(max(0, len(items) - trail), len(items)):
                        def mk_d(j):
                            def f():
                                if items[j][2][-1] >= PCH:
                                    flush_pre(b)
                                emit_av(j)
                            return f
                        deferred.append(mk_d(j))
                    return deferred

                carry = None
                for b in range(B):
                    kT, (vac, van) = nxt[0], nxt[1]
                    # everything batch b's attention reads must be emitted
                    # before its items (emission order = dependency order);
                    # batch 0's v-projection must NOT be force-emitted here
                    # (it waits on the late wv DMA) -- the in-loop flush
                    # before the first new-KV AV covers it
                    if b:
                        flush_pre(b)
                    if b + 1 < B:
                        s_kT, s_kv, _ = enq_setup(b + 1)
                        nxt = (s_kT, s_kv)
                    if b + 2 < B:
                        enq_proj(b + 2)
                    o_sb = osbp.tile([128, QCH, 128], FP32, tag="osb",
                                     name=f"osb{b}")
                    carry = attn_batch(b, kT, vac, van, o_sb,
                                           hook0=k_thunks if b == 0 else None,
                                           carry=carry)
                    if b < B - 1:
                        enq_oproj(b, o_sb)
                # ---- last batch's o_proj: rotate psum through flex +
                # both score buffers, staging copies split between VectorE
                # and ScalarE.  tt0/tt1 (o^T ready mid-batch) go before the
                # trailing AVs; tt2/tt3 after ----
                bl = B - 1
                out_r = out[bass.ts(bl, T), :].rearrange(
                    "(tt p) d -> p tt d", p=128)
                ostl = stage.tile([128, T // 128, D], BF16, tag="ost",
                                  name="ostl")
                k = 0

                def tail_oproj(tts):
                    nonlocal k
                    for tt in tts:
                        tsl = slice(bl * T + tt * 128,
                                    bl * T + (tt + 1) * 128)
                        for nh in range(2):
                            if k % 3 == 0:
                                ps = flexp.tile([128, 512], FP32, tag="flex",
                                                name="flexz")
                            else:
                                ps = scp.tile([128, GCH * 128], FP32,
                                              tag="sc", name="scz")[:, :512]
                            nc.tensor.matmul(
                                ps, lhsT=oT[:, tsl],
                                rhs=woT_s[:, bass.ts(nh, 512)],
                                start=True, stop=True,
                            )
                            if k % 2 == 0:
                                nc.vector.tensor_copy(
                                    ostl[:, tt, bass.ts(nh, 512)], ps)
                            else:
                                nc.scalar.copy(
                                    ostl[:, tt, bass.ts(nh, 512)], ps)
                            if nh == 1:
                                nc.sync.dma_start(out_r[:, tt, :],
                                                  ostl[:, tt, :])
                            k += 1

                tail_oproj([0, 1])
                for f in carry:
                    f()
                tail_oproj([2, 3])
                drain_bg(len(bg_pre) + len(bg_post))

    nc.compile()
    return nc


def _pack_w(wT):
    """[D, 128] -> [128, D//128, 128] bf16 (partition-major for one DMA)."""
    return np.ascontiguousarray(
        wT.reshape(D // 128, 128, 128).transpose(1, 0, 2)
    ).astype(ml_dtypes.bfloat16)


def _pack_v(v):
    """[B, HPC, PAST, HD] -> [B, 128, HPC, PCH, HD+1] bf16, ones in col HD."""
    out = np.empty((B, 128, HPC, PCH, HD + 1), ml_dtypes.bfloat16)
    # v[b, h, c*128 + p, hd] -> out[b, p, h, c, hd]
    out[..., :HD] = v.reshape(B, HPC, PCH, 128, HD).transpose(0, 3, 1, 2, 4)
    out[..., HD] = 1.0
    return np.ascontiguousarray(out)


def _prep(x, k_prev, v_prev, Wq, Wk, Wv, Wo):
    """Host-side shard + layout marshalling."""
    f = np.float32
    bf = ml_dtypes.bfloat16
    x2 = np.ascontiguousarray(np.asarray(x, f).reshape(TOK, D))
    xT = np.ascontiguousarray(x2.T)
    k_prev = np.asarray(k_prev, f)
    v_prev = np.asarray(v_prev, f)
    Wq, Wk, Wv, Wo = (np.asarray(w, f) for w in (Wq, Wk, Wv, Wo))
    in_maps = []
    for c in range(NCORES):
        rows = slice(128 * c, 128 * (c + 1))
        hsl = slice(HPC * c, HPC * (c + 1))
        in_maps.append(
            {
                "xT": xT.astype(bf),
                "wqT": _pack_w((Wq[rows, :] * SCALE).T),
                "wkT": _pack_w(Wk[rows, :].T),
                "wvT": _pack_w(Wv[rows, :].T),
                "woT": np.ascontiguousarray(Wo[:, rows].T),
                "kTp": np.ascontiguousarray(
                    k_prev[:, hsl, :, :].transpose(0, 1, 3, 2)
                ).reshape(B, 128, PAST).astype(bf),
                "vp": _pack_v(v_prev[:, hsl, :, :]),
            }
        )
    return in_maps


def kernel(x, k_prev, v_prev, Wq, Wk, Wv, Wo):
    if "nc" not in _cache:
        _cache["nc"] = _build()
    nc = _cache["nc"]
    in_maps = _prep(x, k_prev, v_prev, Wq, Wk, Wv, Wo)
    res = run_bass_kernel_spmd(nc, in_maps, core_ids=list(range(NCORES)))
    acc = np.zeros((TOK, D), np.float64)
    for r in res.results:
        acc += np.asarray(r["out"], dtype=np.float64)
    return acc.astype(np.float32).reshape(B, T, D)


# revision 34
# speedup vs baseline: 1.0009x; 1.0009x over previous
"""Multi-head attention with KV cache, sharded over 8 NeuronCores by head.

Problem (hardcoded shapes):
  x       [4, 512, 1024]      hidden states (B, T, D)
  k_prev  [4, 16, 3584, 64]   KV cache (B, H, PAST, HD)
  v_prev  [4, 16, 3584, 64]
  Wq/Wk/Wv/Wo [1024, 1024]    projection weights (torch Linear: y = x @ W.T)

Sharding: 16 heads / 8 cores = 2 heads per core (data stays full along batch).
Each core computes q/k/v projections for its 2 heads (column-parallel),
full attention for its heads, and a column-parallel o_proj partial
[2048, 1024]; the host sums the 8 partials (the o_proj all-reduce).

Device algorithm per core:
  ScalarE (exp over every live [key, query] score: ~16M elements/core at
  128 lanes x 1.2 GHz ~= 122 us) is the bottleneck engine, so the whole
  batch loop is one software pipeline that keeps it streaming:
  - work item = (head, query chunk of 128, balanced group of ~10 key
    chunks).  Per item: bf16 scores k @ q^T into a 3-bank PSUM tile (only
    causally-live chunks; the diagonal chunk gets a shared [128,128]
    lower-triangle mask added via identity @ mask), one exp -> bf16 p^T,
    then P @ V accumulation matmuls.
  - P @ V runs transposed: acc[q(128 part), hd+1] accumulates over key
    chunks (65-col matmuls, half the PE column cost of the [hd, q]
    orientation); rhs = v chunk | 1, so the softmax denominator lands in
    col 64 and the divide is a per-partition reciprocal + tensor_scalar.
    The 4 query chunks time-share one PSUM bank as *sequential*
    accumulation groups (the backend allows only one open accumulation
    group per bank - interleaved groups corrupt each other).
  - pipeline beat: exp(i) | scores(i+2) | AV(i-4).  Scores stay 2 tiles
    ahead of ScalarE (scp bufs=2), AV lags 4 items (pT bufs=8) so thunk
    hiccups never stall the exp stream; each batch's trailing AVs are
    carried past the next batch's first two score items.
  - projection (bf16) / KV-append / v-transpose / o_proj work is chopped
    into small thunks drained 2-3 per beat through the PE slack, with
    emission-order barriers before their consumers' batches (thunks must
    be *emitted* before any consumer instruction - program order defines
    dependencies).
  - cold start: PE warmup matmuls ramp the p-state while DMAs land in
    q-path-first order; batch 0 emits q-proj inline, k-proj + cache-append
    at beat 0 between the first two score items, v-proj via beats.
  - tail: last batch's o_proj rotates PSUM through flex + both score
    buffers with staging copies split DVE/ScalarE; out is bf16 (host
    upconverts while summing the 8 o_proj partials in float64).
"""

import numpy as np
import ml_dtypes

import concourse.bass as bass
import concourse.mybir as mybir
import concourse.tile as tile
from concourse import bacc
from concourse.bass_utils import run_bass_kernel_spmd
from concourse.masks import make_identity

B, T, D = 4, 512, 1024
H, HD = 16, 64
PAST = 3584
L = PAST + T            # 4096 == MAX_CACHE, nothing is trimmed
SCALE = 1.0 / np.sqrt(HD).astype(np.float32)
NCORES = 8
HPC = H // NCORES       # heads per core = 2
TOK = B * T             # 2048
NCH = L // 128          # 32 key chunks per (b, h)
PCH = PAST // 128       # 28 chunks from the cache
QCH = T // 128          # 4 query chunks
GCH = 12                # key chunks per PSUM scores group (3 banks)
FP32 = mybir.dt.float32
FP32R = mybir.dt.float32r
BF16 = mybir.dt.bfloat16
NEG = -1.0e30

_cache = {}


def _build():
    nc = bacc.Bacc(None, target_bir_lowering=False)

    xT = nc.dram_tensor("xT", [D, TOK], BF16, kind="ExternalInput")
    wqT = nc.dram_tensor("wqT", [128, D // 128, 128], BF16,
                         kind="ExternalInput")
    wkT = nc.dram_tensor("wkT", [128, D // 128, 128], BF16,
                         kind="ExternalInput")
    wvT = nc.dram_tensor("wvT", [128, D // 128, 128], BF16,
                         kind="ExternalInput")
    woT = nc.dram_tensor("woT", [128, D], FP32R, kind="ExternalInput")
    kTp = nc.dram_tensor("kTp", [B, 128, PAST], BF16, kind="ExternalInput")
    vp = nc.dram_tensor("vp", [B, 128, HPC, PCH, HD + 1], BF16, kind="ExternalInput")
    out = nc.dram_tensor("out", [TOK, D], BF16, kind="ExternalOutput")

    Exp = mybir.ActivationFunctionType.Exp
    mult = mybir.AluOpType.mult

    with tile.TileContext(nc) as tc:
        with (
            tc.tile_pool(name="const", bufs=1) as const,
            tc.tile_pool(name="persist", bufs=1) as persist,
            tc.tile_pool(name="kv", bufs=2) as kv,
            tc.tile_pool(name="pt", bufs=8) as ptp,
            tc.tile_pool(name="div", bufs=2) as divp,
            tc.tile_pool(name="osb", bufs=2) as osbp,
            tc.tile_pool(name="stage", bufs=2) as stage,
            tc.tile_pool(name="acc_ps", bufs=1, space="PSUM") as accp,
            tc.tile_pool(name="flex_ps", bufs=1, space="PSUM") as flexp,
        ):
            # ---- constants ----
            identity = const.tile([128, 128], FP32)
            make_identity(nc, identity)
            identity_b = const.tile([128, 128], BF16)
            nc.vector.tensor_copy(identity_b, identity)
            # shared lower-triangle mask for the diagonal key chunks:
            # tri[key j, query i] = 0 where i >= j else NEG
            tri_f = const.tile([128, 128], FP32)
            nc.gpsimd.memset(tri_f, 0.0)
            nc.gpsimd.affine_select(
                out=tri_f, in_=tri_f, compare_op=mybir.AluOpType.is_ge,
                fill=NEG, base=0, channel_multiplier=-1,
                pattern=[[1, 128]],
            )
            tri_b = const.tile([128, 128], BF16)
            nc.vector.tensor_copy(tri_b, tri_f)

            ones_c = const.tile([128, 1], FP32)
            nc.gpsimd.memset(ones_c, 1.0)
            warm = const.tile([1, 1], FP32)
            nc.scalar.activation(warm, ones_c[:1, :], Exp)

            wrhs = const.tile([128, 512], BF16)
            nc.gpsimd.memset(wrhs, 0.0)

            def pe_warmup(n):
                # dummy back-to-back matmuls on resident constants: ramps
                # the tensor engine's p-state before real work arrives
                wtile = flexp.tile([128, 512], FP32, tag="flex",
                                   name="warmps")
                for i in range(n):
                    nc.tensor.matmul(
                        wtile, lhsT=identity_b, rhs=wrhs,
                        start=True, stop=True,
                        skip_group_check=True,
                    )

            # ---- persistent SBUF ----
            woT_s = persist.tile([128, D], FP32R)
            qT = persist.tile([128, TOK], BF16, tag="qT")
            kTn = persist.tile([128, TOK], BF16, tag="kTn")
            vTn = persist.tile([128, TOK], FP32, tag="vTn")
            oT = persist.tile([128, TOK], FP32R, tag="oT")

            with (
                tc.tile_pool(name="xw", bufs=1) as xw,
                tc.tile_pool(name="xs", bufs=2) as xs,
                tc.tile_pool(name="sc_ps", bufs=2, space="PSUM") as scp,
            ):
                xT_r = xT.rearrange("(ko p) t -> p ko t", p=128)

                # ---- background thunk machinery ----
                # bg_pre: proj/KV-setup thunks; MUST all be emitted before
                # the attention batch that reads their outputs (barrier at
                # attn_batch start).  bg_post: o_proj thunks, no such
                # hazard -- drained through leftover beat slack.
                bg_pre = []      # entries: (due_batch, thunk)
                bg_post = []

                def drain_bg(n=1):
                    for _ in range(n):
                        if bg_pre:
                            bg_pre.pop(0)[1]()
                        elif bg_post:
                            bg_post.pop(0)()

                def flush_pre(due):
                    rest = []
                    for d, f in bg_pre:
                        if d <= due:
                            f()
                        else:
                            rest.append((d, f))
                    bg_pre[:] = rest

                def enq_proj(tcn, xT_s=None):
                    """DMA x chunk now; enqueue 3 proj psum groups as
                    thunks (4x 2-matmul + 1 copy each)."""
                    if xT_s is None:
                        xT_s = xs.tile(
                            [128, D // 128, 512], BF16, tag="xT",
                            name=f"xT_s{tcn}"
                        )
                        half = D // 256
                        nc.sync.dma_start(
                            xT_s[:, :half, :], xT_r[:, :half, bass.ts(tcn, 512)]
                        )
                        nc.sync.dma_start(
                            xT_s[:, half:, :], xT_r[:, half:, bass.ts(tcn, 512)]
                        )
                    for name, dst in (("q", qT), ("k", kTn), ("v", vTn)):
                        box = {}

                        def mk_mm(name, ko, box):
                            def f():
                                if "ps" not in box:
                                    box["ps"] = flexp.tile(
                                        [128, 512], FP32, tag="flex",
                                        name="flexp"
                                    )
                                nc.tensor.matmul(
                                    box["ps"],
                                    lhsT=w_s[name][:, ko, :],
                                    rhs=xT_s[:, ko, :],
                                    start=(ko == 0),
                                    stop=(ko == D // 128 - 1),
                                )
                            return f

                        def mk_cp(dst, box, lo, hi):
                            def f():
                                nc.vector.tensor_copy(
                                    dst[:, tcn * 512 + lo : tcn * 512 + hi],
                                    box["ps"][:, lo:hi]
                                )
                            return f

                        for ko in range(D // 128):
                            bg_pre.append((tcn, mk_mm(name, ko, box)))
                        if tcn == 0 and name in ("q", "k"):
                            # first query chunk lands first: unblocks the
                            # cold-start scores a copy earlier
                            bg_pre.append((tcn, mk_cp(dst, box, 0, 128)))
                            bg_pre.append((tcn, mk_cp(dst, box, 128, 512)))
                        else:
                            bg_pre.append((tcn, mk_cp(dst, box, 0, 512)))

                def enq_setup(b, kT=None, vac=None, inline_kcopy=False):
                    """DMA next batch's KV now; enqueue the cache-append
                    copy + v transposes as thunks."""
                    if kT is None:
                        kT = kv.tile([128, L], BF16, tag="kT", name=f"kT{b}")
                        nc.sync.dma_start(kT[:, :PAST], kTp[b, :, :])
                    if vac is None:
                        vac = kv.tile(
                            [128, HPC, PCH, HD + 1], BF16, tag="vac",
                            name=f"vac{b}"
                        )
                        nc.sync.dma_start(vac, vp[b, :, :, :, :])
                    van = kv.tile([128, HPC, QCH, HD + 1], BF16, tag="van",
                                  name=f"van{b}")
                    def kcopy_a():
                        nc.vector.tensor_copy(
                            kT[:, PAST : PAST + 128],
                            kTn[:, b * T : b * T + 128])

                    def kcopy_b():
                        nc.vector.tensor_copy(
                            kT[:, PAST + 128 :],
                            kTn[:, b * T + 128 : (b + 1) * T])
                    ret_kcopy = None
                    if inline_kcopy == "defer":
                        ret_kcopy = (kcopy_a, kcopy_b)
                    elif inline_kcopy:
                        kcopy_a(); kcopy_b()
                    else:
                        bg_pre.append((b, kcopy_a))
                        bg_pre.append((b, kcopy_b))
                    bg_pre.append((b, lambda: nc.gpsimd.memset(
                        van[:, :, :, HD], 1.0)))
                    for h in range(HPC):
                        hsl = slice(h * HD, (h + 1) * HD)
                        for tt in range(QCH):
                            def mk_tr(hsl, h, tt):
                                def f():
                                    tp = flexp.tile([128, 512], FP32,
                                                    tag="flex", name="flexv")
                                    nc.tensor.transpose(
                                        tp[:, :HD],
                                        vTn[hsl, b * T + tt * 128
                                            : b * T + (tt + 1) * 128],
                                        identity[hsl, hsl],
                                    )
                                    nc.vector.tensor_copy(
                                        van[:, h, tt, :HD], tp[:, :HD]
                                    )
                                return f
                            bg_pre.append((b, mk_tr(hsl, h, tt)))
                    return kT, (vac, van), ret_kcopy

                def enq_oproj(b, o_sb):
                    """o^T transposes + o_proj partial for batch b (drained
                    during batch b+1's pipeline)."""
                    for qq in range(QCH):
                        def mk_tr(qq):
                            def f():
                                tp = flexp.tile([128, 512], FP32, tag="flex",
                                                name="flexo")
                                nc.tensor.transpose(
                                    tp[:, :128], o_sb[:, qq, :], identity
                                )
                                nc.vector.tensor_copy(
                                    oT[:, b * T + qq * 128
                                       : b * T + (qq + 1) * 128],
                                    tp[:, :128],
                                )
                            return f
                        bg_post.append(mk_tr(qq))
                    out_r = out[bass.ts(b, T), :].rearrange(
                        "(tt p) d -> p tt d", p=128)
                    ost = stage.tile([128, T // 128, D], BF16, tag="ost",
                                     name=f"ost{b}")
                    for tt in range(T // 128):
                        tsl = slice(b * T + tt * 128, b * T + (tt + 1) * 128)
                        box = {}

                        def mk_mm(tt, tsl, nh, box):
                            def f():
                                ps = flexp.tile([128, 512], FP32, tag="flex",
                                                name="flexm")
                                nc.tensor.matmul(
                                    ps,
                                    lhsT=oT[:, tsl],
                                    rhs=woT_s[:, bass.ts(nh, 512)],
                                    start=True,
                                    stop=True,
                                )
                                box[nh] = ps
                            return f

                        def mk_cp(tt, nh, box):
                            def f():
                                nc.vector.tensor_copy(
                                    ost[:, tt, bass.ts(nh, 512)], box[nh]
                                )
                                if nh == 1:
                                    nc.sync.dma_start(
                                        out_r[:, tt, :], ost[:, tt, :]
                                    )
                            return f

                        for nh in range(2):
                            bg_post.append(mk_mm(tt, tsl, nh, box))
                            bg_post.append(mk_cp(tt, nh, box))

                # ---- weight / first-batch loads ----
                # DMA order drives the cold start: q path (wq, x chunk 0)
                # first, then k cache + wk so the first exps stream while
                # v / o-weights land
                pe_warmup(8)
                w_s = {}
                for name, w in (("q", wqT), ("k", wkT), ("v", wvT)):
                    w_s[name] = xw.tile(
                        [128, D // 128, 128], BF16, tag=f"w{name}",
                        name=f"w{name}"
                    )
                nc.sync.dma_start(w_s["q"], wqT[:, :, :])
                xT_s0 = xs.tile([128, D // 128, 512], BF16, tag="xT",
                                name="xT_s0")
                half = D // 256
                nc.sync.dma_start(xT_s0[:, :half, :], xT_r[:, :half, :512])
                nc.sync.dma_start(xT_s0[:, half:, :], xT_r[:, half:, :512])
                kT0 = kv.tile([128, L], BF16, tag="kT", name="kT0")
                nc.sync.dma_start(kT0[:, : 12 * 128], kTp[0, :, : 12 * 128])
                nc.sync.dma_start(kT0[:, 12 * 128 : PAST], kTp[0, :, 12 * 128 :])
                nc.sync.dma_start(w_s["k"][:, :4, :], wkT[:, :4, :])
                nc.sync.dma_start(w_s["k"][:, 4:, :], wkT[:, 4:, :])
                nc.sync.dma_start(w_s["v"], wvT[:, :, :])
                vac0 = kv.tile(
                    [128, HPC, PCH, HD + 1], BF16, tag="vac", name="vac0"
                )
                nc.sync.dma_start(vac0[:, 0, :, :], vp[0, :, 0, :, :])
                nc.sync.dma_start(vac0[:, 1, :, :], vp[0, :, 1, :, :])
                nc.sync.dma_start(woT_s, woT[:, :])

                # prologue: q and k projections + cache-append inline (the
                # first scores read them); v projection and the v-transposes
                # drain through batch 0's beats
                enq_proj(0, xT_s=xT_s0)
                # emit only the q projection inline; the k projection +
                # cache-append run at beat 0 of batch 0, AFTER the first two
                # score items, so the first exps start as early as possible
                for _ in range(D // 128 + 2):
                    bg_pre.pop(0)[1]()
                k_thunks = [bg_pre.pop(0)[1] for _ in range(D // 128 + 2)]
                kT0_, kv0, kcopy0 = enq_setup(0, kT=kT0, vac=vac0,
                                              inline_kcopy="defer")
                nxt = (kT0_, kv0)
                # [mms, cp(0:128), kcopy(chunk 28), cp(128:512), kcopy rest]
                k_thunks = (k_thunks[:-1] + [kcopy0[0]]
                            + k_thunks[-1:] + [kcopy0[1]])
                enq_proj(1)

                # ---- the attention pipeline ----
                def attn_batch(b, kT, vac, van, o_sb, hook0=None,
                               carry=None):
                    items = []
                    for h in range(HPC):
                        for qq in range(QCH):
                            nch_q = PCH + qq + 1
                            ngr = (nch_q + GCH - 1) // GCH
                            szs = [nch_q // ngr + (1 if i < nch_q % ngr
                                                   else 0)
                                   for i in range(ngr)]
                            gs, s0 = [], 0
                            for sz in szs:
                                gs.append(list(range(s0, s0 + sz)))
                                s0 += sz
                            for gi, g in enumerate(gs):
                                items.append(
                                    (h, qq, g, gi == len(gs) - 1))
                    ps_t = [None] * len(items)
                    pT_t = [None] * len(items)
                    accs = {}

                    def emit_scores(i):
                        h, qq, g, _ = items[i]
                        hsl = slice(h * HD, (h + 1) * HD)
                        qTh = qT[hsl, b * T + qq * 128
                                 : b * T + (qq + 1) * 128]
                        ps = scp.tile([128, GCH * 128], FP32, tag="sc",
                                      name="scps")
                        ps_t[i] = ps
                        for j, cc in enumerate(g):
                            diag = cc == PCH + qq
                            nc.tensor.matmul(
                                ps[:, bass.ts(j, 128)],
                                lhsT=kT[hsl, bass.ts(cc, 128)],
                                rhs=qTh,
                                start=True,
                                stop=not diag,
                            )
                            if diag:
                                nc.tensor.matmul(
                                    ps[:, bass.ts(j, 128)],
                                    lhsT=identity_b,
                                    rhs=tri_b,
                                    start=False,
                                    stop=True,
                                    skip_group_check=True,
                                )

                    def emit_exp(i):
                        h, qq, g, _ = items[i]
                        ng = len(g)
                        pT = ptp.tile([128, GCH * 128], BF16, tag="pT",
                                      name="pTt")
                        pT_t[i] = pT
                        nc.scalar.activation(
                            pT[:, : ng * 128], ps_t[i][:, : ng * 128], Exp
                        )

                    def emit_av(i):
                        h, qq, g, last = items[i]
                        hosl = slice(h * HD, (h + 1) * HD)
                        if h not in accs:
                            accs[h] = accp.tile([128, QCH, 128], FP32,
                                                tag="acc", name="acct")
                        acc = accs[h]
                        pT = pT_t[i]
                        for j, cc in enumerate(g):
                            vrhs = (vac[:, h, cc, :] if cc < PCH
                                    else van[:, h, cc - PCH, :])
                            nc.tensor.matmul(
                                acc[:, qq, : HD + 1],
                                lhsT=pT[:, bass.ts(j, 128)],
                                rhs=vrhs,
                                start=(cc == 0),
                                stop=(cc == PCH + qq),
                                skip_group_check=True,
                            )
                        if last:
                            r = divp.tile([128, 1], FP32, tag="r", name="rt")
                            nc.vector.reciprocal(
                                r, acc[:, qq, HD : HD + 1])
                            nc.vector.tensor_scalar(
                                out=o_sb[:, qq, hosl],
                                in0=acc[:, qq, :HD],
                                scalar1=r,
                                scalar2=None,
                                op0=mult,
                            )
                            if b == B - 1 and h == HPC - 1:
                                # drain o^T for the final o_proj early
                                tp = flexp.tile([128, 512], FP32,
                                                tag="flex", name="flexq")
                                nc.tensor.transpose(
                                    tp[:, :128], o_sb[:, qq, :], identity)
                                nc.vector.tensor_copy(
                                    oT[:, b * T + qq * 128
                                       : b * T + (qq + 1) * 128],
                                    tp[:, :128])

                    LAG = 4
                    emit_scores(0)
                    if len(items) > 1:
                        emit_scores(1)
                    if carry:
                        for f in carry:
                            f()
                    for i in range(len(items)):
                        emit_exp(i)
                        if i == 0 and hook0:
                            for f in hook0:
                                f()
                        if i + 2 < len(items):
                            emit_scores(i + 2)
                        trail = LAG if b == B - 1 else LAG + 2
                        if i >= LAG and i - LAG < len(items) - trail:
                            j = i - LAG
                            if items[j][2][-1] >= PCH:
                                flush_pre(b)  # AV reads van/kT-append
                            emit_av(j)
                        if i < len(items) - 2:
                            if b == 0:
                                if i >= 1:
                                    drain_bg(3 if len(bg_pre) > 25 else 2)
                            else:
                                drain_bg(2)
                    deferred = []
                    for j in range(max(0, len(items) - trail), len(items)):
                        def mk_d(j):
                            def f():
                                if items[j][2][-1] >= PCH:
                                    flush_pre(b)
                                emit_av(j)
                            return f
                        deferred.append(mk_d(j))
                    return deferred

                carry = None
                for b in range(B):
                    kT, (vac, van) = nxt[0], nxt[1]
                    # everything batch b's attention reads must be emitted
                    # before its items (emission order = dependency order);
                    # batch 0's v-projection must NOT be force-emitted here
                    # (it waits on the late wv DMA) -- the in-loop flush
                    # before the first new-KV AV covers it
                    if b:
                        flush_pre(b)
                    if b + 1 < B:
                        s_kT, s_kv, _ = enq_setup(b + 1)
                        nxt = (s_kT, s_kv)
                    if b + 2 < B:
                        enq_proj(b + 2)
                    o_sb = osbp.tile([128, QCH, 128], FP32, tag="osb",
                                     name=f"osb{b}")
                    carry = attn_batch(b, kT, vac, van, o_sb,
                                           hook0=k_thunks if b == 0 else None,
                                           carry=carry)
                    if b < B - 1:
                        enq_oproj(b, o_sb)
                # ---- last batch's o_proj: rotate psum through flex +
                # both score buffers, staging copies split between VectorE
                # and ScalarE.  tt0/tt1 (o^T ready mid-batch) go before the
                # trailing AVs; tt2/tt3 after ----
                bl = B - 1
                out_r = out[bass.ts(bl, T), :].rearrange(
                    "(tt p) d -> p tt d", p=128)
                ostl = stage.tile([128, T // 128, D], BF16, tag="ost",
                                  name="ostl")
                k = 0

                def tail_oproj(tts):
                    nonlocal k
                    for tt in tts:
                        tsl = slice(bl * T + tt * 128,
                                    bl * T + (tt + 1) * 128)
                        for nh in range(2):
                            if k % 3 == 0:
                                ps = flexp.tile([128, 512], FP32, tag="flex",
                                                name="flexz")
                            else:
                                ps = scp.tile([128, GCH * 128], FP32,
                                              tag="sc", name="scz")[:, :512]
                            nc.tensor.matmul(
                                ps, lhsT=oT[:, tsl],
                                rhs=woT_s[:, bass.ts(nh, 512)],
                                start=True, stop=True,
                            )
                            if k % 2 == 0:
                                nc.vector.tensor_copy(
                                    ostl[:, tt, bass.ts(nh, 512)], ps)
                            else:
                                nc.scalar.copy(
                                    ostl[:, tt, bass.ts(nh, 512)], ps)
                            if nh == 1:
                                nc.sync.dma_start(out_r[:, tt, :],
                                                  ostl[:, tt, :])
                            k += 1

                tail_oproj([0, 1])
                for f in carry:
                    f()
                tail_oproj([2, 3])
                drain_bg(len(bg_pre) + len(bg_post))

    nc.compile()
    return nc


def _pack_w(wT):
    """[D, 128] -> [128, D//128, 128] bf16 (partition-major for one DMA)."""
    return np.ascontiguousarray(
        wT.reshape(D // 128, 128, 128).transpose(1, 0, 2)
    ).astype(ml_dtypes.bfloat16)


def _pack_v(v):
    """[B, HPC, PAST, HD] -> [B, 128, HPC, PCH, HD+1] bf16, ones in col HD."""
    out = np.empty((B, 128, HPC, PCH, HD + 1), ml_dtypes.bfloat16)
    # v[b, h, c*128 + p, hd] -> out[b, p, h, c, hd]
    out[..., :HD] = v.reshape(B, HPC, PCH, 128, HD).transpose(0, 3, 1, 2, 4)
    out[..., HD] = 1.0
    return np.ascontiguousarray(out)


def _prep(x, k_prev, v_prev, Wq, Wk, Wv, Wo):
    """Host-side shard + layout marshalling."""
    f = np.float32
    bf = ml_dtypes.bfloat16
    x2 = np.ascontiguousarray(np.asarray(x, f).reshape(TOK, D))
    xT = np.ascontiguousarray(x2.T)
    k_prev = np.asarray(k_prev, f)
    v_prev = np.asarray(v_prev, f)
    Wq, Wk, Wv, Wo = (np.asarray(w, f) for w in (Wq, Wk, Wv, Wo))
    in_maps = []
    for c in range(NCORES):
        rows = slice(128 * c, 128 * (c + 1))
        hsl = slice(HPC * c, HPC * (c + 1))
        in_maps.append(
            {
                "xT": xT.astype(bf),
                "wqT": _pack_w((Wq[rows, :] * SCALE).T),
                "wkT": _pack_w(Wk[rows, :].T),
                "wvT": _pack_w(Wv[rows, :].T),
                "woT": np.ascontiguousarray(Wo[:, rows].T),
                "kTp": np.ascontiguousarray(
                    k_prev[:, hsl, :, :].transpose(0, 1, 3, 2)
                ).reshape(B, 128, PAST).astype(bf),
                "vp": _pack_v(v_prev[:, hsl, :, :]),
            }
        )
    return in_maps


def kernel(x, k_prev, v_prev, Wq, Wk, Wv, Wo):
    if "nc" not in _cache:
        _cache["nc"] = _build()
    nc = _cache["nc"]
    in_maps = _prep(x, k_prev, v_prev, Wq, Wk, Wv, Wo)
    res = run_bass_kernel_spmd(nc, in_maps, core_ids=list(range(NCORES)))
    acc = np.zeros((TOK, D), np.float64)
    for r in res.results:
        acc += np.asarray(r["out"], dtype=np.float64)
    return acc.astype(np.float32).reshape(B, T, D)


# revision 35
# speedup vs baseline: 1.0015x; 1.0007x over previous
"""Multi-head attention with KV cache, sharded over 8 NeuronCores by head.

Problem (hardcoded shapes):
  x       [4, 512, 1024]      hidden states (B, T, D)
  k_prev  [4, 16, 3584, 64]   KV cache (B, H, PAST, HD)
  v_prev  [4, 16, 3584, 64]
  Wq/Wk/Wv/Wo [1024, 1024]    projection weights (torch Linear: y = x @ W.T)

Sharding: 16 heads / 8 cores = 2 heads per core (data stays full along batch).
Each core computes q/k/v projections for its 2 heads (column-parallel),
full attention for its heads, and a column-parallel o_proj partial
[2048, 1024]; the host sums the 8 partials (the o_proj all-reduce).

Device algorithm per core:
  ScalarE (exp over every live [key, query] score: ~16M elements/core at
  128 lanes x 1.2 GHz ~= 122 us) is the bottleneck engine, so the whole
  batch loop is one software pipeline that keeps it streaming:
  - work item = (head, query chunk of 128, balanced group of ~10 key
    chunks).  Per item: bf16 scores k @ q^T into a 3-bank PSUM tile (only
    causally-live chunks; the diagonal chunk gets a shared [128,128]
    lower-triangle mask added via identity @ mask), one exp -> bf16 p^T,
    then P @ V accumulation matmuls.
  - P @ V runs transposed: acc[q(128 part), hd+1] accumulates over key
    chunks (65-col matmuls, half the PE column cost of the [hd, q]
    orientation); rhs = v chunk | 1, so the softmax denominator lands in
    col 64 and the divide is a per-partition reciprocal + tensor_scalar.
    The 4 query chunks time-share one PSUM bank as *sequential*
    accumulation groups (the backend allows only one open accumulation
    group per bank - interleaved groups corrupt each other).
  - pipeline beat: exp(i) | scores(i+2) | AV(i-4).  Scores stay 2 tiles
    ahead of ScalarE (scp bufs=2), AV lags 4 items (pT bufs=8) so thunk
    hiccups never stall the exp stream; each batch's trailing AVs are
    carried past the next batch's first two score items.
  - projection (bf16) / KV-append / v-transpose / o_proj work is chopped
    into small thunks drained 2-3 per beat through the PE slack, with
    emission-order barriers before their consumers' batches (thunks must
    be *emitted* before any consumer instruction - program order defines
    dependencies).
  - cold start: PE warmup matmuls ramp the p-state while DMAs land in
    q-path-first order; batch 0 emits q-proj inline, k-proj + cache-append
    at beat 0 between the first two score items, v-proj via beats.
  - tail: last batch's o_proj rotates PSUM through flex + both score
    buffers with staging copies split DVE/ScalarE; out is bf16 (host
    upconverts while summing the 8 o_proj partials in float64).
"""

import numpy as np
import ml_dtypes

import concourse.bass as bass
import concourse.mybir as mybir
import concourse.tile as tile
from concourse import bacc
from concourse.bass_utils import run_bass_kernel_spmd
from concourse.masks import make_identity

B, T, D = 4, 512, 1024
H, HD = 16, 64
PAST = 3584
L = PAST + T            # 4096 == MAX_CACHE, nothing is trimmed
SCALE = 1.0 / np.sqrt(HD).astype(np.float32)
NCORES = 8
HPC = H // NCORES       # heads per core = 2
TOK = B * T             # 2048
NCH = L // 128          # 32 key chunks per (b, h)
PCH = PAST // 128       # 28 chunks from the cache
QCH = T // 128          # 4 query chunks
GCH = 12                # key chunks per PSUM scores group (3 banks)
FP32 = mybir.dt.float32
FP32R = mybir.dt.float32r
BF16 = mybir.dt.bfloat16
NEG = -1.0e30

_cache = {}


def _build():
    nc = bacc.Bacc(None, target_bir_lowering=False)

    xT = nc.dram_tensor("xT", [D, TOK], BF16, kind="ExternalInput")
    wqT = nc.dram_tensor("wqT", [128, D // 128, 128], BF16,
                         kind="ExternalInput")
    wkT = nc.dram_tensor("wkT", [128, D // 128, 128], BF16,
                         kind="ExternalInput")
    wvT = nc.dram_tensor("wvT", [128, D // 128, 128], BF16,
                         kind="ExternalInput")
    woT = nc.dram_tensor("woT", [128, D], FP32R, kind="ExternalInput")
    kTp = nc.dram_tensor("kTp", [B, 128, PAST], BF16, kind="ExternalInput")
    vp = nc.dram_tensor("vp", [B, 128, HPC, PCH, HD + 1], BF16, kind="ExternalInput")
    out = nc.dram_tensor("out", [TOK, D], BF16, kind="ExternalOutput")

    Exp = mybir.ActivationFunctionType.Exp
    mult = mybir.AluOpType.mult

    with tile.TileContext(nc) as tc:
        with (
            tc.tile_pool(name="const", bufs=1) as const,
            tc.tile_pool(name="persist", bufs=1) as persist,
            tc.tile_pool(name="kv", bufs=2) as kv,
            tc.tile_pool(name="pt", bufs=8) as ptp,
            tc.tile_pool(name="div", bufs=2) as divp,
            tc.tile_pool(name="osb", bufs=2) as osbp,
            tc.tile_pool(name="stage", bufs=2) as stage,
            tc.tile_pool(name="acc_ps", bufs=1, space="PSUM") as accp,
            tc.tile_pool(name="flex_ps", bufs=1, space="PSUM") as flexp,
        ):
            # ---- constants ----
            identity = const.tile([128, 128], FP32)
            make_identity(nc, identity)
            identity_b = const.tile([128, 128], BF16)
            nc.vector.tensor_copy(identity_b, identity)
            # shared lower-triangle mask for the diagonal key chunks:
            # tri[key j, query i] = 0 where i >= j else NEG
            tri_f = const.tile([128, 128], FP32)
            nc.gpsimd.memset(tri_f, 0.0)
            nc.gpsimd.affine_select(
                out=tri_f, in_=tri_f, compare_op=mybir.AluOpType.is_ge,
                fill=NEG, base=0, channel_multiplier=-1,
                pattern=[[1, 128]],
            )
            tri_b = const.tile([128, 128], BF16)
            nc.vector.tensor_copy(tri_b, tri_f)

            ones_c = const.tile([128, 1], FP32)
            nc.gpsimd.memset(ones_c, 1.0)
            warm = const.tile([1, 1], FP32)
            nc.scalar.activation(warm, ones_c[:1, :], Exp)

            wrhs = const.tile([128, 512], BF16)
            nc.gpsimd.memset(wrhs, 0.0)

            def pe_warmup(n):
                # dummy back-to-back matmuls on resident constants: ramps
                # the tensor engine's p-state before real work arrives
                wtile = flexp.tile([128, 512], FP32, tag="flex",
                                   name="warmps")
                for i in range(n):
                    nc.tensor.matmul(
                        wtile, lhsT=identity_b, rhs=wrhs,
                        start=True, stop=True,
                        skip_group_check=True,
                    )

            # ---- persistent SBUF ----
            woT_s = persist.tile([128, D], FP32R)
            qT = persist.tile([128, TOK], BF16, tag="qT")
            kTn = persist.tile([128, TOK], BF16, tag="kTn")
            vTn = persist.tile([128, TOK], FP32, tag="vTn")
            oT = persist.tile([128, TOK], FP32R, tag="oT")

            with (
                tc.tile_pool(name="xw", bufs=1) as xw,
                tc.tile_pool(name="xs", bufs=2) as xs,
                tc.tile_pool(name="sc_ps", bufs=2, space="PSUM") as scp,
            ):
                xT_r = xT.rearrange("(ko p) t -> p ko t", p=128)

                # ---- background thunk machinery ----
                # bg_pre: proj/KV-setup thunks; MUST all be emitted before
                # the attention batch that reads their outputs (barrier at
                # attn_batch start).  bg_post: o_proj thunks, no such
                # hazard -- drained through leftover beat slack.
                bg_pre = []      # entries: (due_batch, thunk)
                bg_post = []

                def drain_bg(n=1):
                    for _ in range(n):
                        if bg_pre:
                            bg_pre.pop(0)[1]()
                        elif bg_post:
                            bg_post.pop(0)()

                def flush_pre(due):
                    rest = []
                    for d, f in bg_pre:
                        if d <= due:
                            f()
                        else:
                            rest.append((d, f))
                    bg_pre[:] = rest

                def enq_proj(tcn, xT_s=None):
                    """DMA x chunk now; enqueue 3 proj psum groups as
                    thunks (4x 2-matmul + 1 copy each)."""
                    if xT_s is None:
                        xT_s = xs.tile(
                            [128, D // 128, 512], BF16, tag="xT",
                            name=f"xT_s{tcn}"
                        )
                        half = D // 256
                        nc.sync.dma_start(
                            xT_s[:, :half, :], xT_r[:, :half, bass.ts(tcn, 512)]
                        )
                        nc.sync.dma_start(
                            xT_s[:, half:, :], xT_r[:, half:, bass.ts(tcn, 512)]
                        )
                    for name, dst in (("q", qT), ("k", kTn), ("v", vTn)):
                        box = {}

                        def mk_mm(name, ko, box):
                            def f():
                                if "ps" not in box:
                                    box["ps"] = flexp.tile(
                                        [128, 512], FP32, tag="flex",
                                        name="flexp"
                                    )
                                nc.tensor.matmul(
                                    box["ps"],
                                    lhsT=w_s[name][:, ko, :],
                                    rhs=xT_s[:, ko, :],
                                    start=(ko == 0),
                                    stop=(ko == D // 128 - 1),
                                )
                            return f

                        def mk_cp(dst, box, lo, hi):
                            def f():
                                nc.vector.tensor_copy(
                                    dst[:, tcn * 512 + lo : tcn * 512 + hi],
                                    box["ps"][:, lo:hi]
                                )
                            return f

                        for ko in range(D // 128):
                            bg_pre.append((tcn, mk_mm(name, ko, box)))
                        if tcn == 0 and name in ("q", "k"):
                            # first query chunk lands first: unblocks the
                            # cold-start scores a copy earlier
                            bg_pre.append((tcn, mk_cp(dst, box, 0, 128)))
                            bg_pre.append((tcn, mk_cp(dst, box, 128, 512)))
                        else:
                            bg_pre.append((tcn, mk_cp(dst, box, 0, 512)))

                def enq_setup(b, kT=None, vac=None, inline_kcopy=False):
                    """DMA next batch's KV now; enqueue the cache-append
                    copy + v transposes as thunks."""
                    if kT is None:
                        kT = kv.tile([128, L], BF16, tag="kT", name=f"kT{b}")
                        nc.sync.dma_start(kT[:, :PAST], kTp[b, :, :])
                    if vac is None:
                        vac = kv.tile(
                            [128, HPC, PCH, HD + 1], BF16, tag="vac",
                            name=f"vac{b}"
                        )
                        nc.sync.dma_start(vac, vp[b, :, :, :, :])
                    van = kv.tile([128, HPC, QCH, HD + 1], BF16, tag="van",
                                  name=f"van{b}")
                    def kcopy_a():
                        nc.vector.tensor_copy(
                            kT[:, PAST : PAST + 128],
                            kTn[:, b * T : b * T + 128])

                    def kcopy_b():
                        nc.vector.tensor_copy(
                            kT[:, PAST + 128 :],
                            kTn[:, b * T + 128 : (b + 1) * T])
                    ret_kcopy = None
                    if inline_kcopy == "defer":
                        ret_kcopy = (kcopy_a, kcopy_b)
                    elif inline_kcopy:
                        kcopy_a(); kcopy_b()
                    else:
                        bg_pre.append((b, kcopy_a))
                        bg_pre.append((b, kcopy_b))
                    bg_pre.append((b, lambda: nc.gpsimd.memset(
                        van[:, :, :, HD], 1.0)))
                    for h in range(HPC):
                        hsl = slice(h * HD, (h + 1) * HD)
                        for tt in range(QCH):
                            def mk_tr(hsl, h, tt):
                                def f():
                                    tp = flexp.tile([128, 512], FP32,
                                                    tag="flex", name="flexv")
                                    nc.tensor.transpose(
                                        tp[:, :HD],
                                        vTn[hsl, b * T + tt * 128
                                            : b * T + (tt + 1) * 128],
                                        identity[hsl, hsl],
                                    )
                                    nc.vector.tensor_copy(
                                        van[:, h, tt, :HD], tp[:, :HD]
                                    )
                                return f
                            bg_pre.append((b, mk_tr(hsl, h, tt)))
                    return kT, (vac, van), ret_kcopy

                def enq_oproj(b, o_sb):
                    """o^T transposes + o_proj partial for batch b (drained
                    during batch b+1's pipeline)."""
                    for qq in range(QCH):
                        def mk_tr(qq):
                            def f():
                                tp = flexp.tile([128, 512], FP32, tag="flex",
                                                name="flexo")
                                nc.tensor.transpose(
                                    tp[:, :128], o_sb[:, qq, :], identity
                                )
                                nc.vector.tensor_copy(
                                    oT[:, b * T + qq * 128
                                       : b * T + (qq + 1) * 128],
                                    tp[:, :128],
                                )
                            return f
                        bg_post.append(mk_tr(qq))
                    out_r = out[bass.ts(b, T), :].rearrange(
                        "(tt p) d -> p tt d", p=128)
                    ost = stage.tile([128, T // 128, D], BF16, tag="ost",
                                     name=f"ost{b}")
                    for tt in range(T // 128):
                        tsl = slice(b * T + tt * 128, b * T + (tt + 1) * 128)
                        box = {}

                        def mk_mm(tt, tsl, nh, box):
                            def f():
                                ps = flexp.tile([128, 512], FP32, tag="flex",
                                                name="flexm")
                                nc.tensor.matmul(
                                    ps,
                                    lhsT=oT[:, tsl],
                                    rhs=woT_s[:, bass.ts(nh, 512)],
                                    start=True,
                                    stop=True,
                                )
                                box[nh] = ps
                            return f

                        def mk_cp(tt, nh, box):
                            def f():
                                nc.vector.tensor_copy(
                                    ost[:, tt, bass.ts(nh, 512)], box[nh]
                                )
                                if nh == 1:
                                    nc.sync.dma_start(
                                        out_r[:, tt, :], ost[:, tt, :]
                                    )
                            return f

                        for nh in range(2):
                            bg_post.append(mk_mm(tt, tsl, nh, box))
                            bg_post.append(mk_cp(tt, nh, box))

                # ---- weight / first-batch loads ----
                # DMA order drives the cold start: q path (wq, x chunk 0)
                # first, then k cache + wk so the first exps stream while
                # v / o-weights land
                pe_warmup(8)
                w_s = {}
                for name, w in (("q", wqT), ("k", wkT), ("v", wvT)):
                    w_s[name] = xw.tile(
                        [128, D // 128, 128], BF16, tag=f"w{name}",
                        name=f"w{name}"
                    )
                nc.sync.dma_start(w_s["q"], wqT[:, :, :])
                xT_s0 = xs.tile([128, D // 128, 512], BF16, tag="xT",
                                name="xT_s0")
                half = D // 256
                nc.sync.dma_start(xT_s0[:, :half, :], xT_r[:, :half, :512])
                nc.sync.dma_start(xT_s0[:, half:, :], xT_r[:, half:, :512])
                kT0 = kv.tile([128, L], BF16, tag="kT", name="kT0")
                nc.sync.dma_start(kT0[:, : 12 * 128], kTp[0, :, : 12 * 128])
                nc.sync.dma_start(kT0[:, 12 * 128 : PAST], kTp[0, :, 12 * 128 :])
                nc.sync.dma_start(w_s["k"][:, :4, :], wkT[:, :4, :])
                nc.sync.dma_start(w_s["k"][:, 4:, :], wkT[:, 4:, :])
                nc.sync.dma_start(w_s["v"], wvT[:, :, :])
                vac0 = kv.tile(
                    [128, HPC, PCH, HD + 1], BF16, tag="vac", name="vac0"
                )
                nc.sync.dma_start(vac0[:, 0, :, :], vp[0, :, 0, :, :])
                nc.sync.dma_start(vac0[:, 1, :, :], vp[0, :, 1, :, :])
                nc.sync.dma_start(woT_s, woT[:, :])

                # prologue: q and k projections + cache-append inline (the
                # first scores read them); v projection and the v-transposes
                # drain through batch 0's beats
                enq_proj(0, xT_s=xT_s0)
                # emit only the q projection inline; the k projection +
                # cache-append run at beat 0 of batch 0, AFTER the first two
                # score items, so the first exps start as early as possible
                for _ in range(D // 128 + 2):
                    bg_pre.pop(0)[1]()
                k_thunks = [bg_pre.pop(0)[1] for _ in range(D // 128 + 2)]
                kT0_, kv0, kcopy0 = enq_setup(0, kT=kT0, vac=vac0,
                                              inline_kcopy="defer")
                nxt = (kT0_, kv0)
                # [mms, cp(0:128), kcopy(chunk 28), cp(128:512), kcopy rest]
                k_thunks = (k_thunks[:-1] + [kcopy0[0]]
                            + k_thunks[-1:] + [kcopy0[1]])
                enq_proj(1)

                # ---- the attention pipeline ----
                def attn_batch(b, kT, vac, van, o_sb, hook0=None,
                               carry=None):
                    items = []
                    for h in range(HPC):
                        for qq in range(QCH):
                            nch_q = PCH + qq + 1
                            ngr = (nch_q + GCH - 1) // GCH
                            szs = [nch_q // ngr + (1 if i < nch_q % ngr
                                                   else 0)
                                   for i in range(ngr)]
                            gs, s0 = [], 0
                            for sz in szs:
                                gs.append(list(range(s0, s0 + sz)))
                                s0 += sz
                            for gi, g in enumerate(gs):
                                items.append(
                                    (h, qq, g, gi == len(gs) - 1))
                    ps_t = [None] * len(items)
                    pT_t = [None] * len(items)
                    accs = {}

                    def emit_scores(i):
                        h, qq, g, _ = items[i]
                        hsl = slice(h * HD, (h + 1) * HD)
                        qTh = qT[hsl, b * T + qq * 128
                                 : b * T + (qq + 1) * 128]
                        ps = scp.tile([128, GCH * 128], FP32, tag="sc",
                                      name="scps")
                        ps_t[i] = ps
                        for j, cc in enumerate(g):
                            diag = cc == PCH + qq
                            nc.tensor.matmul(
                                ps[:, bass.ts(j, 128)],
                                lhsT=kT[hsl, bass.ts(cc, 128)],
                                rhs=qTh,
                                start=True,
                                stop=not diag,
                            )
                            if diag:
                                nc.tensor.matmul(
                                    ps[:, bass.ts(j, 128)],
                                    lhsT=identity_b,
                                    rhs=tri_b,
                                    start=False,
                                    stop=True,
                                    skip_group_check=True,
                                )

                    def emit_exp(i):
                        h, qq, g, _ = items[i]
                        ng = len(g)
                        pT = ptp.tile([128, GCH * 128], BF16, tag="pT",
                                      name="pTt")
                        pT_t[i] = pT
                        nc.scalar.activation(
                            pT[:, : ng * 128], ps_t[i][:, : ng * 128], Exp
                        )

                    def emit_av(i):
                        h, qq, g, last = items[i]
                        hosl = slice(h * HD, (h + 1) * HD)
                        if h not in accs:
                            accs[h] = accp.tile([128, QCH, 128], FP32,
                                                tag="acc", name="acct")
                        acc = accs[h]
                        pT = pT_t[i]
                        for j, cc in enumerate(g):
                            vrhs = (vac[:, h, cc, :] if cc < PCH
                                    else van[:, h, cc - PCH, :])
                            nc.tensor.matmul(
                                acc[:, qq, : HD + 1],
                                lhsT=pT[:, bass.ts(j, 128)],
                                rhs=vrhs,
                                start=(cc == 0),
                                stop=(cc == PCH + qq),
                                skip_group_check=True,
                            )
                        if last:
                            r = divp.tile([128, 1], FP32, tag="r", name="rt")
                            nc.vector.reciprocal(
                                r, acc[:, qq, HD : HD + 1])
                            nc.vector.tensor_scalar(
                                out=o_sb[:, qq, hosl],
                                in0=acc[:, qq, :HD],
                                scalar1=r,
                                scalar2=None,
                                op0=mult,
                            )
                            if b == B - 1 and h == HPC - 1:
                                # drain o^T for the final o_proj early
                                tp = flexp.tile([128, 512], FP32,
                                                tag="flex", name="flexq")
                                nc.tensor.transpose(
                                    tp[:, :128], o_sb[:, qq, :], identity)
                                nc.vector.tensor_copy(
                                    oT[:, b * T + qq * 128
                                       : b * T + (qq + 1) * 128],
                                    tp[:, :128])

                    LAG = 4
                    emit_scores(0)
                    if len(items) > 1:
                        emit_scores(1)
                    if carry:
                        for f in carry:
                            f()
                    for i in range(len(items)):
                        emit_exp(i)
                        if i == 0 and hook0:
                            for f in hook0:
                                f()
                        if i + 2 < len(items):
                            emit_scores(i + 2)
                        trail = LAG if b == B - 1 else LAG + 2
                        if i >= LAG and i - LAG < len(items) - trail:
                            j = i - LAG
                            if items[j][2][-1] >= PCH:
                                flush_pre(b)  # AV reads van/kT-append
                            emit_av(j)
                        if i < len(items) - 2:
                            if b == 0:
                                # beats 1-3 have no AV work yet (LAG):
                                # front-load the v-proj/setup drains there
                                if 1 <= i <= 3:
                                    drain_bg(4)
                                elif i >= 4:
                                    drain_bg(3 if len(bg_pre) > 25 else 2)
                            else:
                                drain_bg(2)
                    deferred = []
                    for j in range(max(0, len(items) - trail), len(items)):
                        def mk_d(j):
                            def f():
                                if items[j][2][-1] >= PCH:
                                    flush_pre(b)
                                emit_av(j)
                            return f
                        deferred.append(mk_d(j))
                    return deferred

                carry = None
                for b in range(B):
                    kT, (vac, van) = nxt[0], nxt[1]
                    # everything batch b's attention reads must be emitted
                    # before its items (emission order = dependency order);
                    # batch 0's v-projection must NOT be force-emitted here
                    # (it waits on the late wv DMA) -- the in-loop flush
                    # before the first new-KV AV covers it
                    if b:
                        flush_pre(b)
                    if b + 1 < B:
                        s_kT, s_kv, _ = enq_setup(b + 1)
                        nxt = (s_kT, s_kv)
                    if b + 2 < B:
                        enq_proj(b + 2)
                    o_sb = osbp.tile([128, QCH, 128], FP32, tag="osb",
                                     name=f"osb{b}")
                    carry = attn_batch(b, kT, vac, van, o_sb,
                                           hook0=k_thunks if b == 0 else None,
                                           carry=carry)
                    if b < B - 1:
                        enq_oproj(b, o_sb)
                # ---- last batch's o_proj: rotate psum through flex +
                # both score buffers, staging copies split between VectorE
                # and ScalarE.  tt0/tt1 (o^T ready mid-batch) go before the
                # trailing AVs; tt2/tt3 after ----
                bl = B - 1
                out_r = out[bass.ts(bl, T), :].rearrange(
                    "(tt p) d -> p tt d", p=128)
                ostl = stage.tile([128, T // 128, D], BF16, tag="ost",
                                  name="ostl")
                k = 0

                def tail_oproj(tts):
                    nonlocal k
                    for tt in tts:
                        tsl = slice(bl * T + tt * 128,
                                    bl * T + (tt + 1) * 128)
                        for nh in range(2):
                            if k % 3 == 0:
                                ps = flexp.tile([128, 512], FP32, tag="flex",
                                                name="flexz")
                            else:
                                ps = scp.tile([128, GCH * 128], FP32,
                                              tag="sc", name="scz")[:, :512]
                            nc.tensor.matmul(
                                ps, lhsT=oT[:, tsl],
                                rhs=woT_s[:, bass.ts(nh, 512)],
                                start=True, stop=True,
                            )
                            if k % 2 == 0:
                                nc.vector.tensor_copy(
                                    ostl[:, tt, bass.ts(nh, 512)], ps)
                            else:
                                nc.scalar.copy(
                                    ostl[:, tt, bass.ts(nh, 512)], ps)
                            if nh == 1:
                                nc.sync.dma_start(out_r[:, tt, :],
                                                  ostl[:, tt, :])
                            k += 1

                tail_oproj([0, 1])
                for f in carry:
                    f()
                tail_oproj([2, 3])
                drain_bg(len(bg_pre) + len(bg_post))

    nc.compile()
    return nc


def _pack_w(wT):
    """[D, 128] -> [128, D//128, 128] bf16 (partition-major for one DMA)."""
    return np.ascontiguousarray(
        wT.reshape(D // 128, 128, 128).transpose(1, 0, 2)
    ).astype(ml_dtypes.bfloat16)


def _pack_v(v):
    """[B, HPC, PAST, HD] -> [B, 128, HPC, PCH, HD+1] bf16, ones in col HD."""
    out = np.empty((B, 128, HPC, PCH, HD + 1), ml_dtypes.bfloat16)
    # v[b, h, c*128 + p, hd] -> out[b, p, h, c, hd]
    out[..., :HD] = v.reshape(B, HPC, PCH, 128, HD).transpose(0, 3, 1, 2, 4)
    out[..., HD] = 1.0
    return np.ascontiguousarray(out)


def _prep(x, k_prev, v_prev, Wq, Wk, Wv, Wo):
    """Host-side shard + layout marshalling."""
    f = np.float32
    bf = ml_dtypes.bfloat16
    x2 = np.ascontiguousarray(np.asarray(x, f).reshape(TOK, D))
    xT = np.ascontiguousarray(x2.T)
    k_prev = np.asarray(k_prev, f)
    v_prev = np.asarray(v_prev, f)
    Wq, Wk, Wv, Wo = (np.asarray(w, f) for w in (Wq, Wk, Wv, Wo))
    in_maps = []
    for c in range(NCORES):
        rows = slice(128 * c, 128 * (c + 1))
        hsl = slice(HPC * c, HPC * (c + 1))
        in_maps.append(
            {
                "xT": xT.astype(bf),
                "wqT": _pack_w((Wq[rows, :] * SCALE).T),
                "wkT": _pack_w(Wk[rows, :].T),
                "wvT": _pack_w(Wv[rows, :].T),
                "woT": np.ascontiguousarray(Wo[:, rows].T),
                "kTp": np.ascontiguousarray(
                    k_prev[:, hsl, :, :].transpose(0, 1, 3, 2)
                ).reshape(B, 128, PAST).astype(bf),
                "vp": _pack_v(v_prev[:, hsl, :, :]),
            }
        )
    return in_maps


def kernel(x, k_prev, v_prev, Wq, Wk, Wv, Wo):
    if "nc" not in _cache:
        _cache["nc"] = _build()
    nc = _cache["nc"]
    in_maps = _prep(x, k_prev, v_prev, Wq, Wk, Wv, Wo)
    res = run_bass_kernel_spmd(nc, in_maps, core_ids=list(range(NCORES)))
    acc = np.zeros((TOK, D), np.float64)
    for r in res.results:
        acc += np.asarray(r["out"], dtype=np.float64)
    return acc.astype(np.float32).reshape(B, T, D)
